# revision 2
# baseline (speedup 1.0000x reference)
"""Trainium2 Bass kernel for GNN message passing (8 NeuronCores, SPMD) — v3.

    out = segment_sum(x[src] @ W, tgt, N) + x @ W_self

Identity: segment_sum(x[src] @ W, tgt) = segment_sum(x[src], tgt) @ W, so the
per-edge matmul hoists out of the reduction.

Cost structure on this silicon (measured via pipelined R-slope microbench):
a dma_gather costs ~1 us fixed + ~4.3 ns/idx of Q7 descriptor-generation,
executed by the single core pair (2q, 2q+1) owning its queue; instructions on
different queues overlap ~2.6x when the WAR horizon (G pool depth) allows.
The SDMA drain itself is cheap. So the gather plan is:
  - one call per (3-window group, chunk): 768 idxs/call, under the ~1008
    descriptor-ring cap (64 descs/engine) that wedges bigger calls;
  - queue = chunk index with num_swdge_queues=4, so the 4 calls of a group
    land on all four Q7 core pairs concurrently;
  - gpool bufs=6 keeps the WAR dependency ~6 groups ahead so the POOL
    sequencer never blocks on gather consumers;
  - pads gather row 0 of their chunk (safe, DRAM-page-hit cheap). The
    ucode's trailing-negative skip is NOT used: it desyncs the persistent
    DGE ring bookkeeping across executions (decode pushes pre-skip counts,
    the Q7 writes post-skip) and wedges the second run.

Everything else: bf16 end-to-end (gather 256 B/row, one-hot S via DVE
2x_1P is_equal with materialized iota, matmuls bf16 with fp32 PSUM, bf16
output converted on host), hT/outT copies on ACT, xT preloaded, W-apply
grouped at N=512.
"""

import numpy as np

P = 128
D = 128
N_NODES = 100000
N_CORES = 8
N_LOC = N_NODES // N_CORES          # 12500
N_WIN = (N_LOC + P - 1) // P        # 98
N_PAD = N_WIN * P                   # 12544

# dma_gather uses int16 row indices, so x is addressed through 4 overlapping
# 32768-row chunks; every source row is reachable from >=1 chunk and rows in
# overlap regions can go to either side, which lets the host balance the four
# per-window runs under the per-chunk tile cap.
N_CHUNK = 4
CHUNK_SPAN = 32768
CHUNK_BASE = [0, 22411, 44822, N_NODES - CHUNK_SPAN]

G_WIN = 3                           # windows per gather group (98 = 32*3 + 2)

_program_cache: dict = {}


def _cap_g_win(g_win, t_c):
    # per-call idx cap ~1008 (64-desc/engine SWDGE ring); keep calls under it
    return min(g_win, max(1, 1008 // (t_c * P)))


def _group_sizes(g_win):
    sizes = [g_win] * (N_WIN // g_win)
    if N_WIN % g_win:
        sizes.append(N_WIN % g_win)
    return sizes


def _build_program(
    t_win: int,
    reps: int = 1,
    n_queues: int = 4,
    w_group: int = 4,
    g_win: int = G_WIN,
    g_bufs: int = 6,
    split_idx: bool = True,
):
    import concourse.mybir as mybir
    import concourse.tile as tile
    from concourse.bacc import Bacc

    f32 = mybir.dt.float32
    bf16 = mybir.dt.bfloat16
    t_c = t_win // N_CHUNK
    g_win = _cap_g_win(g_win, t_c)
    sizes = _group_sizes(g_win)

    # consts packed as [idx16 | tl (bf16) | iota_nt (bf16) | W | Ws]
    idx_cols16 = N_WIN * N_CHUNK * t_c * 8          # int16 columns
    idx_cols = idx_cols16 // 2                      # as int32 columns
    t_tot = N_WIN * t_win
    tl_cols = t_tot // 2                            # bf16 -> int32 columns
    iota_cols = t_win * P // 2
    w_cols = D // 2
    k_const = idx_cols + tl_cols + iota_cols + 2 * w_cols

    # Bacc (not raw Bass): its finalize() legalizes sync waits — TRN2 allows
    # at most one semaphore wait per instruction and walrus rejects more.
    nc = Bacc(num_swdge_queues=n_queues)
    xb_d = nc.declare_dram_parameter("xb", [N_NODES, D], bf16, isOutput=False)
    xT_d = nc.declare_dram_parameter("xT", [D, N_PAD], bf16, isOutput=False)
    consts_d = nc.declare_dram_parameter(
        "consts", [P, k_const], mybir.dt.int32, isOutput=False
    )
    # bf16 output halves the store stream; host converts back to fp32
    # (adds ~2^-9 relative rounding, well inside the 2e-2 gate)
    outT_d = nc.declare_dram_parameter("outT", [D, N_PAD], bf16, isOutput=True)

    with tile.TileContext(nc) as tc:
        with (
            tc.tile_pool(name="const", bufs=1) as cpool,
            tc.tile_pool(name="gath", bufs=g_bufs) as gpool,
            tc.tile_pool(name="spool", bufs=4) as spool,
            tc.tile_pool(name="wtile", bufs=3) as wpool,
            tc.tile_pool(name="psum", bufs=2, space="PSUM") as psum,
            tc.tile_pool(name="opsum", bufs=2, space="PSUM") as opsum,
        ):
            if split_idx:
                # idx16 in its own tile + DMA: gathers depend only on it, so
                # they start before the (larger) tl/iota/W/xT streams land.
                idx_t = cpool.tile([P, idx_cols], mybir.dt.int32)
                # head piece (first 8 groups) lands fast so the first
                # gathers start before the bulk idx stream arrives
                head = min(idx_cols, 8 * N_CHUNK * g_win * t_c * 8 // 2)
                nc.sync.dma_start(idx_t[:, :head], consts_d[:, :head])
                nc.sync.dma_start(idx_t[:, head:], consts_d[:, head:idx_cols])
                rest_t = cpool.tile([P, k_const - idx_cols], mybir.dt.int32)
                nc.sync.dma_start(rest_t[:], consts_d[:, idx_cols:])
                idx16_sb = idx_t[:].bitcast(mybir.dt.int16)
                o = 0
            else:
                rest_t = cpool.tile([P, k_const], mybir.dt.int32)
                nc.sync.dma_start(rest_t[:], consts_d[:])
                idx16_sb = rest_t[:, :idx_cols].bitcast(mybir.dt.int16)
                o = idx_cols
            xT_sb = cpool.tile([D, N_PAD], bf16)
            nc.sync.dma_start(xT_sb[:], xT_d[:])

            tl_sb = rest_t[:, o : o + tl_cols].bitcast(bf16)
            o += tl_cols
            iota_sb = rest_t[:, o : o + iota_cols].bitcast(bf16)
            o += iota_cols
            w_sb = rest_t[:, o : o + w_cols].bitcast(bf16)
            o += w_cols
            ws_sb = rest_t[:, o : o + w_cols].bitcast(bf16)
            # iota_nt[p, n, t] = n  (constant along t) — materialized so the
            # is_equal's in0 has inner step 1 (2x_1P eligibility).
            iota_nt = iota_sb.rearrange("p (n t) -> p n t", t=t_win)

            for rep in range(reps):
                w0 = 0
                for grp, g_sz in enumerate(sizes):
                    # one gather call per chunk covers all g_sz windows:
                    # G_big[:, c, wl*t_c + i, :] = 128 gathered rows (bf16)
                    # for window w0+wl, chunk c, tile i.
                    G_big = gpool.tile([P, N_CHUNK, g_win * t_c, D], bf16, tag="G")
                    for c in range(N_CHUNK):
                        nidx = g_sz * t_c * P
                        cw0 = (w0 * N_CHUNK + g_sz * c) * (t_c * 8)
                        nc.gpsimd.dma_gather(
                            G_big[:, c, 0 : g_sz * t_c, :],
                            xb_d[CHUNK_BASE[c] : CHUNK_BASE[c] + CHUNK_SPAN, :],
                            idx16_sb[:, cw0 : cw0 + g_sz * t_c * 8],
                            nidx,
                            nidx,
                            D,
                            queue_num=c % n_queues,
                        )
                    for wl in range(g_sz):
                        w = w0 + wl
                        hT_ps = psum.tile([D, P], f32)
                        # S[p, n, t] one-hot: tl broadcast along n (outer,
                        # step 0), inner t step 1 -> DVE 2x_1P.
                        S_big = spool.tile([P, P, t_win], bf16)
                        nc.vector.tensor_tensor(
                            out=S_big[:],
                            in0=iota_nt,
                            in1=tl_sb[
                                :, None, w * t_win : (w + 1) * t_win
                            ].to_broadcast([P, P, t_win]),
                            op=mybir.AluOpType.is_equal,
                        )
                        for t in range(t_win):
                            c, i = divmod(t, t_c)
                            nc.tensor.matmul(
                                hT_ps[:],
                                lhsT=G_big[:, c, wl * t_c + i, :],
                                rhs=S_big[:, :, t],
                                start=(t == 0),
                                stop=(t == t_win - 1),
                            )
                        # grouped W-apply: stage hT of w_group windows side by
                        # side (bf16), then stream both weight matmuls at
                        # N = w_group*128.
                        gi = w % w_group
                        if gi == 0:
                            n_in_grp = min(w_group, N_WIN - w)
                            hT_sb = wpool.tile([D, w_group * P], bf16, tag="hT")
                        nc.scalar.copy(hT_sb[:, gi * P : (gi + 1) * P], hT_ps[:])
                        if gi == n_in_grp - 1:
                            wg0 = w - gi
                            span = n_in_grp * P
                            outT_ps = opsum.tile([D, w_group * P], f32)
                            nc.tensor.matmul(
                                outT_ps[:, :span],
                                lhsT=w_sb,
                                rhs=hT_sb[:, :span],
                                start=True,
                                stop=False,
                            )
                            nc.tensor.matmul(
                                outT_ps[:, :span],
                                lhsT=ws_sb,
                                rhs=xT_sb[:, wg0 * P : wg0 * P + span],
                                start=False,
                                stop=True,
                            )
                            o_sb = wpool.tile([D, w_group * P], bf16, tag="o")
                            nc.scalar.copy(o_sb[:, :span], outT_ps[:, :span])
                            nc.sync.dma_start(
                                outT_d[:, wg0 * P : wg0 * P + span],
                                o_sb[:, :span],
                            )
                    w0 += g_sz

    nc.finalize()
    return nc


def _prep_inputs(x, edge_index, W, W_self, g_win=G_WIN, neg_pads=False):
    """Host-side sharding: bucket+sort edges by target core/window, pad to a
    uniform tile count, build per-core input maps."""
    import ml_dtypes

    x = np.ascontiguousarray(np.asarray(x, dtype=np.float32))
    W = np.ascontiguousarray(np.asarray(W, dtype=np.float32))
    W_self = np.ascontiguousarray(np.asarray(W_self, dtype=np.float32))
    ei = np.asarray(edge_index)
    src = ei[0].astype(np.int64)
    tgt = ei[1].astype(np.int64)

    order = np.argsort(tgt, kind="stable")
    src_s = src[order].astype(np.int64)
    tgt_s = tgt[order]
    core = tgt_s // N_LOC
    wloc = (tgt_s - core * N_LOC) // P
    gw = (core * N_WIN + wloc).astype(np.int64)
    counts = np.bincount(gw, minlength=N_CORES * N_WIN)
    t_win_data = max(1, int(np.ceil(counts.max() / P)))
    t_c = max(2, (t_win_data + N_CHUNK - 1) // N_CHUNK)

    # chunk feasibility per edge: lo = highest chunk with base <= s,
    # hi = lowest chunk with s < base + CHUNK_SPAN (consecutive range)
    bases = np.asarray(CHUNK_BASE, np.int64)
    lo = np.searchsorted(bases, src_s, side="right") - 1
    hi = np.searchsorted(bases + CHUNK_SPAN, src_s, side="right")
    starts = np.concatenate([[0], np.cumsum(counts)])
    tl_val = (tgt_s - (core * N_LOC + wloc * P)).astype(np.float32)

    while True:
        cap = t_c * P
        t_win = N_CHUNK * t_c
        t_tot = N_WIN * t_win
        # idx_all[core, w, c, slot]; run_n[core, w, c] = real edge count
        idx_all = np.zeros((N_CORES, N_WIN, N_CHUNK, cap), np.int16)
        run_n = np.zeros((N_CORES, N_WIN, N_CHUNK), np.int32)
        tl_flat = np.full(N_CORES * t_tot * P, -1.0, np.float32)
        ok = True
        for g in range(N_CORES * N_WIN):
            a, b = starts[g], starts[g + 1]
            if b - a > N_CHUNK * cap:
                ok = False
                break
            s_g, hi_g, lo_g, tl_g = src_s[a:b], hi[a:b], lo[a:b], tl_val[a:b]
            taken = np.zeros(b - a, bool)
            c_core, w = divmod(g, N_WIN)
            for c in range(N_CHUNK):
                cand = (~taken) & (hi_g <= c) & (c <= lo_g)
                must = cand & (lo_g == c)
                n_must = int(must.sum())
                if n_must > cap:
                    ok = False
                    break
                sel = must.nonzero()[0]
                flex = (cand & ~must).nonzero()[0][: cap - n_must]
                pick = np.concatenate([sel, flex])
                taken[pick] = True
                n = pick.size
                idx_all[c_core, w, c, :n] = (s_g[pick] - bases[c]).astype(np.int16)
                run_n[c_core, w, c] = n
                # tl slots for this chunk run (pads stay -1)
                base_slot = g * (t_win * P) + c * cap
                tl_flat[base_slot : base_slot + n] = tl_g[pick]
            if not ok or not taken.all():
                ok = ok and bool(taken.all())
                if not ok:
                    break
        if ok:
            break
        t_c += 1

    g_win = _cap_g_win(g_win, t_c)
    sizes = _group_sizes(g_win)
    # tl as bf16 [P, t_tot] per core (slot (w, c, i, p) -> col w*t_win + c*t_c+i)
    tl_dev = np.ascontiguousarray(
        tl_flat.reshape(N_CORES, t_tot, P).transpose(0, 2, 1)
    ).astype(ml_dtypes.bfloat16)
    # iota_nt[p, n*t_win + t] = n (constant along t), identical per partition
    iota_nt = np.tile(
        np.repeat(np.arange(P, dtype=np.float32), t_win).astype(ml_dtypes.bfloat16),
        (P, 1),
    )
    x_b = x.astype(ml_dtypes.bfloat16)
    W_b = W.astype(ml_dtypes.bfloat16)
    Ws_b = W_self.astype(ml_dtypes.bfloat16)
    in_maps = []
    for cc in range(N_CORES):
        # per gather call (grp, c): slots of the group's windows' chunk-c runs
        # concatenated in window order. The call's trailing pads (last
        # window's run tail) are -1 so the ucode skips them; interior pads
        # stay 0 (gather row 0 — safe). Wrapped int16 layout within the
        # call: slot s -> [s % 16, s // 16], 16-partition block replicated
        # to all 8 Q7 core stripes.
        cols = []
        w0 = 0
        for g_sz in sizes:
            for c in range(N_CHUNK):
                blk = idx_all[cc, w0 : w0 + g_sz, c, :].copy()  # [g_sz, cap]
                if neg_pads:
                    n_last = run_n[cc, w0 + g_sz - 1, c]
                    blk[g_sz - 1, n_last:] = -1
                flat = blk.reshape(-1)
                cols.append(flat.reshape(-1, 16))
            w0 += g_sz
        wrapped = np.concatenate(cols, axis=0)        # [total/16, 16]
        sb = np.tile(wrapped.T.reshape(16, -1), (8, 1))
        xT_c = np.zeros((D, N_PAD), np.float32)
        xT_c[:, :N_LOC] = x[cc * N_LOC : (cc + 1) * N_LOC].T
        consts = np.concatenate(
            [
                sb.view(np.int32),
                tl_dev[cc].view(np.int32),
                iota_nt.view(np.int32),
                W_b.view(np.int32),
                Ws_b.view(np.int32),
            ],
            axis=1,
        )
        in_maps.append(
            {
                "xb": x_b,
                "xT": xT_c.astype(ml_dtypes.bfloat16),
                "consts": consts,
            }
        )
    return in_maps, t_win


def run(x, edge_index, W, W_self, trace=False, **trace_kwargs):
    """Returns (output [100000,128] float32, BassKernelResults)."""
    from concourse import bass_utils

    in_maps, t_win = _prep_inputs(x, edge_index, W, W_self)
    nc = _program_cache.get(t_win)
    if nc is None:
        nc = _build_program(t_win)
        _program_cache[t_win] = nc
    # A NeuronCore occasionally comes up wedged from a previous session
    # (NRT_EXEC_UNIT_UNRECOVERABLE); the failed attempt itself clears it, so
    # one retry recovers.
    try:
        res = bass_utils.run_bass_kernel_spmd(
            nc, in_maps, core_ids=list(range(N_CORES)), trace=trace, **trace_kwargs
        )
    except Exception:
        res = bass_utils.run_bass_kernel_spmd(
            nc, in_maps, core_ids=list(range(N_CORES)), trace=trace, **trace_kwargs
        )
    out = np.empty((N_NODES, D), np.float32)
    for c in range(N_CORES):
        out[c * N_LOC : (c + 1) * N_LOC] = (
            res.results[c]["outT"].astype(np.float32).T[:N_LOC]
        )
    return out, res


def kernel(x, edge_index, W, W_self):
    out, _ = run(x, edge_index, W, W_self, trace=False)
    return out



# revision 3
# speedup vs baseline: 1.0619x; 1.0619x over previous
"""Trainium2 Bass kernel for GNN message passing — v6 (shared-target aligned
ragged tiling).

    out = segment_sum(x[src] @ W, tgt, N) + x @ W_self
       = segment_sum(x[src], tgt) @ W + x @ W_self

v3 padded every target window to a uniform t_win=8 tiles (1024 edge slots vs
816 avg) because the gather/S/matmul structure was window-uniform. The kernel
is entirely Q7 descriptor-generation bound (~2.12 ns/idx aggregate over the 4
SWDGE queues, measured), so slot count is the cost. v4 packs each gather call
(group of G_WIN windows x chunk) with the windows' runs CONCATENATED, padding
only to the call's 128-slot tile boundary, with a cross-core max profile so
all 8 cores share one SPMD program:

  - call (g,c) has m_gc = max_core ceil(L_gc/128) tiles; per-core idx arrays
    are 0-padded (row-0 gathers, tl=-1) to the shared profile;
  - window boundaries fall mid-tile and differ per core; each window's matmul
    covers the tile span [a,b] = min/max over cores of its run's tiles, with
    per-core tl = -1 masking foreign slots (S column = 0);
  - a boundary tile is consumed by both adjacent windows with complementary
    masks (one extra 128^3 matmul per boundary; PE has headroom).

v6 on top: chunks 0..2 of every non-last window in a group get a SHARED
(cross-core) run-length target that is a multiple of 128 (pads 0-idx/-1-tl);
chunk 3 absorbs each window's remainder. Aligned boundaries are identical on
all cores, so boundary tiles belong to exactly one window and use-spans don't
widen. ~96k slots/core, ~859 tile-use matmuls; the per-window throwaway sync
matmul is dropped (Bacc legalizes the multi-wait matmuls). Measured 218.8us
vs v3's 231.4us (5v25 R-slope).
"""

import numpy as np

P = 128
D = 128
N_NODES = 100000
N_CORES = 8
N_LOC = N_NODES // N_CORES          # 12500
N_WIN = (N_LOC + P - 1) // P        # 98
N_PAD = N_WIN * P                   # 12544

N_CHUNK = 4
CHUNK_SPAN = 32768
CHUNK_BASE = [0, 22411, 44822, N_NODES - CHUNK_SPAN]

G_WIN = 3
MAX_CALL_TILES = 7                  # 896 idx < 1008 SWDGE ring cap

_program_cache: dict = {}


def _group_sizes(g_win=G_WIN):
    sizes = [g_win] * (N_WIN // g_win)
    if N_WIN % g_win:
        sizes.append(N_WIN % g_win)
    return sizes


def _build_program(profile, reps: int = 1, w_group: int = 4, g_bufs: int = 8,
                   psum_bufs: int = 4, spool_bufs: int = 6, use_scratch: bool = False):
    """profile: dict with
    m_gc[g][c]: tiles per (group, chunk) call;
    uses[w]: list of (c, j) tile coords for window w's matmuls (j local to
             the (g,c) call region);
    u_max: max len(uses[w]);
    tl_off[w]: column offset of window w's tl block (each use = 128 slots,
             but tl is stored one bf16 column of 128 partitions per use);
    """
    import concourse.mybir as mybir
    import concourse.tile as tile
    from concourse.bacc import Bacc

    f32 = mybir.dt.float32
    bf16 = mybir.dt.bfloat16

    m_gc = profile["m_gc"]
    uses = profile["uses"]
    u_max = profile["u_max"]
    sizes = _group_sizes()
    n_groups = len(sizes)

    # per-(g,c) idx16 column offsets (128 idx = 8 int16 cols per tile)
    idx_off = {}
    off = 0
    for g in range(n_groups):
        for c in range(N_CHUNK):
            idx_off[(g, c)] = off
            off += m_gc[g][c] * 8
    idx_cols16 = off
    idx_cols = idx_cols16 // 2

    n_uses_tot = sum(len(u) for u in uses)
    tl_cols = (n_uses_tot + 1) // 2          # bf16 cols -> int32 cols
    iota_cols = u_max * P // 2
    w_cols = D // 2
    k_const = idx_cols + tl_cols + iota_cols + 2 * w_cols

    nc = Bacc(num_swdge_queues=N_CHUNK)
    xb_d = nc.declare_dram_parameter("xb", [N_NODES, D], bf16, isOutput=False)
    xT_d = nc.declare_dram_parameter("xT", [D, N_PAD], bf16, isOutput=False)
    consts_d = nc.declare_dram_parameter(
        "consts", [P, k_const], mybir.dt.int32, isOutput=False
    )
    outT_d = nc.declare_dram_parameter("outT", [D, N_PAD], bf16, isOutput=True)

    with tile.TileContext(nc) as tc:
        with (
            tc.tile_pool(name="const", bufs=1) as cpool,
            tc.tile_pool(name="gath", bufs=g_bufs) as gpool,
            tc.tile_pool(name="spool", bufs=spool_bufs) as spool,
            tc.tile_pool(name="wtile", bufs=3) as wpool,
            tc.tile_pool(name="psum", bufs=psum_bufs, space="PSUM") as psum,
            tc.tile_pool(name="opsum", bufs=2, space="PSUM") as opsum,
            tc.tile_pool(name="scratch", bufs=1, space="PSUM") as scratch_pool,
        ):
            scratch_ps = scratch_pool.tile([1, 1], f32)
            idx_t = cpool.tile([P, idx_cols], mybir.dt.int32)
            head16 = idx_off[(min(8, n_groups - 1), 0)]
            head = head16 // 2
            if head > 0:
                nc.sync.dma_start(idx_t[:, :head], consts_d[:, :head])
                nc.sync.dma_start(idx_t[:, head:], consts_d[:, head:idx_cols])
            else:
                nc.sync.dma_start(idx_t[:], consts_d[:, :idx_cols])
            rest_t = cpool.tile([P, k_const - idx_cols], mybir.dt.int32)
            nc.sync.dma_start(rest_t[:], consts_d[:, idx_cols:])
            idx16_sb = idx_t[:].bitcast(mybir.dt.int16)
            xT_sb = cpool.tile([D, N_PAD], bf16)
            nc.sync.dma_start(xT_sb[:], xT_d[:])

            o = 0
            tl_sb = rest_t[:, o : o + tl_cols].bitcast(bf16)
            o += tl_cols
            iota_sb = rest_t[:, o : o + iota_cols].bitcast(bf16)
            o += iota_cols
            w_sb = rest_t[:, o : o + w_cols].bitcast(bf16)
            o += w_cols
            ws_sb = rest_t[:, o : o + w_cols].bitcast(bf16)
            # iota_nt[p, n, u] = n (constant along u)
            iota_nt = iota_sb.rearrange("p (n u) -> p n u", u=u_max)

            tl_off = profile["tl_off"]

            for rep in range(reps):
                w0 = 0
                for g, g_sz in enumerate(sizes):
                    m_g = sum(m_gc[g])
                    # tile j of chunk c lives at G_big[:, goff[c] + j, :]
                    goff = np.cumsum([0] + list(m_gc[g]))[:-1]
                    G_big = gpool.tile([P, m_g, D], bf16, tag="G")
                    for c in range(N_CHUNK):
                        mt = m_gc[g][c]
                        for t0 in range(0, mt, MAX_CALL_TILES):
                            t1 = min(t0 + MAX_CALL_TILES, mt)
                            nidx = (t1 - t0) * P
                            c0 = idx_off[(g, c)] + t0 * 8
                            nc.gpsimd.dma_gather(
                                G_big[:, goff[c] + t0 : goff[c] + t1, :],
                                xb_d[CHUNK_BASE[c] : CHUNK_BASE[c] + CHUNK_SPAN, :],
                                idx16_sb[:, c0 : c0 + (t1 - t0) * 8],
                                nidx,
                                nidx,
                                D,
                                queue_num=c % N_CHUNK,
                            )
                    for wl in range(g_sz):
                        w = w0 + wl
                        w_uses = uses[w]
                        nu = len(w_uses)
                        hT_ps = psum.tile([D, P], f32)
                        S_big = spool.tile([P, P, u_max], bf16, tag="S")
                        nc.vector.tensor_tensor(
                            out=S_big[:, :, 0:nu],
                            in0=iota_nt[:, :, 0:nu],
                            in1=tl_sb[
                                :, None, tl_off[w] : tl_off[w] + nu
                            ].to_broadcast([P, P, nu]),
                            op=mybir.AluOpType.is_equal,
                        )
                        # 1x1 throwaway matmul: makes the PE queue observe the
                        # DVE tick first so each real matmul carries one wait
                        if use_scratch:
                            nc.tensor.matmul(
                                scratch_ps[:],
                                lhsT=S_big[:, 0, 0:1],
                                rhs=S_big[:, 0, 0:1],
                                start=True,
                                stop=True,
                            )
                        for u, (c, j) in enumerate(w_uses):
                            nc.tensor.matmul(
                                hT_ps[:],
                                lhsT=G_big[:, goff[c] + j, :],
                                rhs=S_big[:, :, u],
                                start=(u == 0),
                                stop=(u == nu - 1),
                            )
                        gi = w % w_group
                        if gi == 0:
                            n_in_grp = min(w_group, N_WIN - w)
                            hT_sb = wpool.tile([D, w_group * P], bf16, tag="hT")
                        nc.scalar.copy(hT_sb[:, gi * P : (gi + 1) * P], hT_ps[:])
                        if gi == n_in_grp - 1:
                            wg0 = w - gi
                            span = n_in_grp * P
                            outT_ps = opsum.tile([D, w_group * P], f32)
                            nc.tensor.matmul(
                                outT_ps[:, :span],
                                lhsT=w_sb,
                                rhs=hT_sb[:, :span],
                                start=True,
                                stop=False,
                            )
                            nc.tensor.matmul(
                                outT_ps[:, :span],
                                lhsT=ws_sb,
                                rhs=xT_sb[:, wg0 * P : wg0 * P + span],
                                start=False,
                                stop=True,
                            )
                            o_sb = wpool.tile([D, w_group * P], bf16, tag="o")
                            nc.scalar.copy(o_sb[:, :span], outT_ps[:, :span])
                            nc.sync.dma_start(
                                outT_d[:, wg0 * P : wg0 * P + span],
                                o_sb[:, :span],
                            )
                    w0 += g_sz

    nc.finalize()
    return nc


def _prep_inputs(x, edge_index, W, W_self):
    """Host-side: sort edges by target window, balance chunks per group,
    build the shared cross-core profile + per-core const tensors."""
    import ml_dtypes

    x = np.ascontiguousarray(np.asarray(x, dtype=np.float32))
    W = np.ascontiguousarray(np.asarray(W, dtype=np.float32))
    W_self = np.ascontiguousarray(np.asarray(W_self, dtype=np.float32))
    ei = np.asarray(edge_index)
    src = ei[0].astype(np.int64)
    tgt = ei[1].astype(np.int64)

    order = np.argsort(tgt, kind="stable")
    src_s = src[order]
    tgt_s = tgt[order]
    core = tgt_s // N_LOC
    wloc = (tgt_s - core * N_LOC) // P
    gw = (core * N_WIN + wloc).astype(np.int64)
    counts = np.bincount(gw, minlength=N_CORES * N_WIN)
    starts = np.concatenate([[0], np.cumsum(counts)])
    tl_val = (tgt_s - (core * N_LOC + wloc * P)).astype(np.float32)

    bases = np.asarray(CHUNK_BASE, np.int64)
    # feasible chunk range [hi, lo] (consecutive), as in v3
    lo = np.searchsorted(bases, src_s, side="right") - 1
    hi = np.searchsorted(bases + CHUNK_SPAN, src_s, side="right")

    sizes = _group_sizes()
    n_groups = len(sizes)

    # ---- per (core, group): assign edges to chunks, balanced ----
    # run_idx[core][g][c] = int16 idx array (concatenated windows, in window
    # order); run_tl[core][g][c] = matching tl floats;
    # run_bounds[core][g][c] = cumulative slot starts per window (len g_sz+1)
    run_idx = [[[None] * N_CHUNK for _ in range(n_groups)] for _ in range(N_CORES)]
    run_tl = [[[None] * N_CHUNK for _ in range(n_groups)] for _ in range(N_CORES)]
    run_bounds = np.zeros((N_CORES, n_groups, N_CHUNK, G_WIN + 1), np.int64)

    gstarts = np.concatenate([[0], np.cumsum(sizes)])
    # Shared-target alignment: for every window except the last of its group,
    # chunks 0..2 get a SHARED (cross-core) run length that is a multiple of
    # 128. Each core fills must-edges + carried flex + pulled flex, then pads
    # with 0-idx/-1-tl. Aligned boundaries are identical on all 8 cores, so
    # boundary tiles are consumed by exactly one window and use-spans don't
    # widen. Chunk 3 takes each window's remainder (ragged).
    for g, g_sz in enumerate(sizes):
        w_lo = gstarts[g]
        # pass 1: per (core, window): must edge lists per chunk, flex per pair
        musts = [[None] * g_sz for _ in range(N_CORES)]
        flexs = [[None] * g_sz for _ in range(N_CORES)]
        for cc in range(N_CORES):
            for wl in range(g_sz):
                gidx = cc * N_WIN + w_lo + wl
                a, b = starts[gidx], starts[gidx + 1]
                s_g = src_s[a:b]
                hi_g, lo_g = hi[a:b], lo[a:b]
                t_g = tl_val[a:b]
                is_flex = hi_g < lo_g
                musts[cc][wl] = [
                    (s_g[m], t_g[m])
                    for m in [np.where(~is_flex & (np.minimum(hi_g, lo_g) == c))[0]
                              for c in range(N_CHUNK)]
                ]
                flexs[cc][wl] = [
                    (s_g[m], t_g[m])
                    for m in [np.where(is_flex & (hi_g == q))[0] for q in range(3)]
                ]
                # each entry is an (s_array, tl_array) pair; nlen() below
        def nlen(pair):
            return int(pair[0].size)

        # pass 2: shared targets per aligned window
        T = np.zeros((g_sz, 3), np.int64)
        for wl in range(g_sz - 1):
            m0 = np.array([nlen(musts[cc][wl][0]) for cc in range(N_CORES)])
            f0 = np.array([nlen(flexs[cc][wl][0]) for cc in range(N_CORES)])
            T[wl, 0] = 128 * max(1, -(-int(m0.max()) // 128))
            fill1 = np.array(
                [nlen(musts[cc][wl][1]) for cc in range(N_CORES)]
            ) + np.maximum(0, m0 + f0 - T[wl, 0])
            T[wl, 1] = 128 * max(1, -(-int(fill1.max()) // 128))
            f1 = np.array([nlen(flexs[cc][wl][1]) for cc in range(N_CORES)])
            pull1 = np.minimum(f1, T[wl, 1] - fill1)
            fill2 = np.array(
                [nlen(musts[cc][wl][2]) for cc in range(N_CORES)]
            ) + (f1 - pull1)
            T[wl, 2] = 128 * max(1, -(-int(fill2.max()) // 128))
        # pass 3: materialize per-core runs
        for cc in range(N_CORES):
            per_chunk_idx = [[] for _ in range(N_CHUNK)]
            per_chunk_tl = [[] for _ in range(N_CHUNK)]

            def put(c, pair):
                s, t = pair
                per_chunk_idx[c].extend((np.asarray(s) - bases[c]).tolist())
                per_chunk_tl[c].extend(np.asarray(t).tolist())

            def pad(c, n):
                per_chunk_idx[c].extend([0] * n)
                per_chunk_tl[c].extend([-1.0] * n)

            empty = (np.zeros(0, np.int64), np.zeros(0, np.float32))
            for wl in range(g_sz):
                mu, fl = musts[cc][wl], flexs[cc][wl]
                if wl < g_sz - 1:
                    carry = empty
                    for c in range(3):
                        room = int(T[wl, c]) - nlen(mu[c]) - nlen(carry)
                        take = min(nlen(fl[c]), max(0, room))
                        put(c, mu[c])
                        put(c, carry)
                        put(c, (fl[c][0][:take], fl[c][1][:take]))
                        pad(c, int(T[wl, c]) - nlen(mu[c]) - nlen(carry) - take)
                        carry = (fl[c][0][take:], fl[c][1][take:])
                    put(3, mu[3])
                    put(3, carry)
                else:
                    # last window: natural balance toward equal call totals
                    ltot = np.array(
                        [len(per_chunk_idx[c]) for c in range(N_CHUNK)],
                        np.float64,
                    )
                    tgt = (ltot.sum() + sum(nlen(m) for m in mu)
                           + sum(nlen(q) for q in fl)) / N_CHUNK
                    for c in range(N_CHUNK):
                        put(c, mu[c])
                        ltot[c] += nlen(mu[c])
                    for q in range(3):
                        sq, tq = fl[q]
                        for e in range(sq.size):
                            c = q if ltot[q] - tgt <= ltot[q + 1] - tgt else q + 1
                            per_chunk_idx[c].append(int(sq[e]) - int(bases[c]))
                            per_chunk_tl[c].append(float(tq[e]))
                            ltot[c] += 1
                for c in range(N_CHUNK):
                    run_bounds[cc, g, c, wl + 1] = len(per_chunk_idx[c])
            for c in range(N_CHUNK):
                run_idx[cc][g][c] = np.asarray(per_chunk_idx[c], np.int16)
                run_tl[cc][g][c] = np.asarray(per_chunk_tl[c], np.float32)

    # ---- shared profile ----
    m_gc = [[0] * N_CHUNK for _ in range(n_groups)]
    for g in range(n_groups):
        for c in range(N_CHUNK):
            mx = max(len(run_idx[cc][g][c]) for cc in range(N_CORES))
            m_gc[g][c] = max(1, (int(mx) + P - 1) // P)

    # window uses: span of tiles [a,b] over cores for each (w, c)
    uses = []
    for g, g_sz in enumerate(sizes):
        for wl in range(g_sz):
            w_uses = []
            for c in range(N_CHUNK):
                a_t, b_t = None, None
                for cc in range(N_CORES):
                    s0 = run_bounds[cc, g, c, wl]
                    s1 = run_bounds[cc, g, c, wl + 1]
                    if s1 > s0:
                        ta = int(s0 // P)
                        tb = int((s1 - 1) // P)
                        a_t = ta if a_t is None else min(a_t, ta)
                        b_t = tb if b_t is None else max(b_t, tb)
                if a_t is not None:
                    for j in range(a_t, b_t + 1):
                        w_uses.append((c, j))
            uses.append(w_uses)
    u_max = max(len(u) for u in uses)
    tl_off = np.concatenate([[0], np.cumsum([len(u) for u in uses])])[:-1]
    n_uses_tot = int(sum(len(u) for u in uses))

    profile = {
        "m_gc": tuple(tuple(r) for r in m_gc),
        "uses": tuple(tuple(u) for u in uses),
        "u_max": u_max,
        "tl_off": tuple(int(t) for t in tl_off),
    }

    # ---- per-core const tensors ----
    idx_cols16 = sum(m_gc[g][c] * 8 for g in range(n_groups) for c in range(N_CHUNK))
    tl_cols16 = 2 * ((n_uses_tot + 1) // 2)
    iota_nu = np.tile(
        np.repeat(np.arange(P, dtype=np.float32), u_max).astype(ml_dtypes.bfloat16),
        (P, 1),
    )
    x_b = x.astype(ml_dtypes.bfloat16)
    W_b = W.astype(ml_dtypes.bfloat16)
    Ws_b = W_self.astype(ml_dtypes.bfloat16)

    in_maps = []
    for cc in range(N_CORES):
        idx_parts = []
        for g in range(n_groups):
            for c in range(N_CHUNK):
                cap = m_gc[g][c] * P
                arr = np.zeros(cap, np.int16)
                r = run_idx[cc][g][c]
                arr[: len(r)] = r
                idx_parts.append(arr.reshape(-1, 16))
        wrapped = np.concatenate(idx_parts, axis=0)
        idx_sb = np.tile(wrapped.T.reshape(16, -1), (8, 1))  # [128, idx_cols16]

        tl_flat = np.full((n_uses_tot, P), -1.0, np.float32)
        for g, g_sz in enumerate(sizes):
            w_lo = gstarts[g]
            for wl in range(g_sz):
                w = w_lo + wl
                for u, (c, j) in enumerate(profile["uses"][w]):
                    s0 = run_bounds[cc, g, c, wl]
                    s1 = run_bounds[cc, g, c, wl + 1]
                    t_lo = j * P
                    # slots of tile j that belong to this window
                    lo_s = max(s0, t_lo)
                    hi_s = min(s1, t_lo + P)
                    if hi_s > lo_s:
                        vals = run_tl[cc][g][c][lo_s:hi_s]
                        tl_flat[tl_off[w] + u, lo_s - t_lo : hi_s - t_lo] = vals
        # [P, n_uses] with pad column to even count
        tl_sb = np.full((P, tl_cols16), -1.0, np.float32)
        tl_sb[:, :n_uses_tot] = tl_flat.T
        tl_sb = tl_sb.astype(ml_dtypes.bfloat16)

        xT_c = np.zeros((D, N_PAD), np.float32)
        xT_c[:, :N_LOC] = x[cc * N_LOC : (cc + 1) * N_LOC].T
        consts = np.concatenate(
            [
                idx_sb.view(np.int32),
                tl_sb.view(np.int32),
                iota_nu.view(np.int32),
                W_b.view(np.int32),
                Ws_b.view(np.int32),
            ],
            axis=1,
        )
        in_maps.append(
            {
                "xb": x_b,
                "xT": xT_c.astype(ml_dtypes.bfloat16),
                "consts": consts,
            }
        )
    return in_maps, profile


def run(x, edge_index, W, W_self, trace=False, **trace_kwargs):
    from concourse import bass_utils

    in_maps, profile = _prep_inputs(x, edge_index, W, W_self)
    key = (profile["m_gc"], profile["uses"])
    nc = _program_cache.get(key)
    if nc is None:
        nc = _build_program(profile)
        _program_cache[key] = nc
    try:
        res = bass_utils.run_bass_kernel_spmd(
            nc, in_maps, core_ids=list(range(N_CORES)), trace=trace, **trace_kwargs
        )
    except Exception:
        res = bass_utils.run_bass_kernel_spmd(
            nc, in_maps, core_ids=list(range(N_CORES)), trace=trace, **trace_kwargs
        )
    out = np.empty((N_NODES, D), np.float32)
    for c in range(N_CORES):
        out[c * N_LOC : (c + 1) * N_LOC] = (
            res.results[c]["outT"].astype(np.float32).T[:N_LOC]
        )
    return out, res


def kernel(x, edge_index, W, W_self):
    out, _ = run(x, edge_index, W, W_self, trace=False)
    return out
